# revision 29
# baseline (speedup 1.0000x reference)
"""Trainium2 Bass kernel for nn_Encoder (bag-of-sentences BiGRU + attention + BCE).

Contract: kernel(**inputs) takes the FULL inputs (as in reference.setup_inputs())
and returns (total_loss, logits) matching reference(). Internally shards the bag
axis B=128 across 8 NeuronCores (16 bags = 320 sentences each), replicates the
small params + embedding table, runs one SPMD Bass program, and gathers.

Design notes (per core):
- token order s-major: flat token index = s*T + t, s in [0,320), t in [0,70).
- x^T [60, TOK] bf16 built on device: word rows gathered from the 400k-row
  embedding table via indirect DMA (128 rows/call), PE-transposed; pos rows are
  host-prefetched payload (tiny 120x5 table) riding the same transpose.
- GRU state/hidden layout: hid 200 split in 2 chunks of 100 partitions;
  all per-step tensors are [100, 2, 320]. Gate preacts accumulate in PSUM:
  x-part matmul + 2 h-part matmuls (bf16, fp32 psum). sigma/tanh fold the biases
  (per-partition ACT bias). n-gate: DVE scalar_tensor_tensor RMW on psum
  computes r*(hn+b_hn) in place, then the xn matmul accumulates on top.
- h' = n + z*(h-n): 3 bf16 tensor_tensor ops (2x DVE mode).
- tup (=hf+hb) stored [100, 2, 70, 320] bf16; word attention: tanh chunks,
  M=1 scores matmuls, softmax in [s-part, t] layout via SBUF->SBUF DMA reshape,
  alpha partition-broadcast, TT-mult + tensor_reduce pooling.
"""
import os
import sys

import numpy as np

sys.path.insert(0, "/opt/trn_rl_repo")

import ml_dtypes

BF16 = ml_dtypes.bfloat16

# problem sizes
VOCAB, WORD_D, POS_N, POS_D = 400000, 50, 120, 5
HID, T, REL, B, NSENT = 200, 70, 100, 128, 20
IN_D = WORD_D + 2 * POS_D  # 60
NCORES = 8
BAGS = B // NCORES         # 16 bags per core
S = BAGS * NSENT           # 320 sentences per core
TOK = S * T                # 22400 tokens per core
CH = 100                   # hid chunk (2 chunks of 100)
G3 = 3 * HID


def _build(sizes=None):
    """Build the single-core Bass program. sizes overrides for small-sim tests."""
    from concourse import bacc, bass, mybir, tile
    from concourse.masks import make_identity

    z = dict(S=S, T=T, TOK=TOK, BAGS=BAGS, NSENT=NSENT, REL=REL, V=VOCAB)
    if sizes:
        z.update(sizes)
    sS, sT, sTOK, sBAGS, sNSENT, sREL, sV = (
        z["S"], z["T"], z["TOK"], z["BAGS"], z["NSENT"], z["REL"], z["V"])
    GT = (sTOK + 127) // 128  # gather tiles (last may be partial)
    f32 = mybir.dt.float32
    bf = mybir.dt.bfloat16
    i32 = mybir.dt.int32
    Alu = mybir.AluOpType
    Act = mybir.ActivationFunctionType

    nc = bacc.Bacc(None, target_bir_lowering=False)

    # ---- DRAM I/O ----
    emb_d = nc.dram_tensor("emb_w", [sV, WORD_D], f32, kind="ExternalInput")
    pos_d = nc.dram_tensor("pos_pay", [128, GT * 10], f32, kind="ExternalInput")
    ids_d = nc.dram_tensor("ids", [128, GT], i32, kind="ExternalInput")
    XK = IN_D + 1  # 60 features + constant-1 row (bias row folded into W_ih)
    wih_d = {d: nc.dram_tensor(f"wih_{d}", [XK, G3], bf, kind="ExternalInput") for d in "fb"}
    whh_d = {d: nc.dram_tensor(f"whh_{d}", [CH, 2 * G3], bf, kind="ExternalInput") for d in "fb"}
    bhn_d = {d: nc.dram_tensor(f"bhn_{d}", [CH, 2], f32, kind="ExternalInput") for d in "fb"}
    aw_d = nc.dram_tensor("aw", [CH, 2], bf, kind="ExternalInput")
    sena_d = nc.dram_tensor("sena", [CH, 2], f32, kind="ExternalInput")
    senr_d = nc.dram_tensor("senr", [CH, 2], bf, kind="ExternalInput")
    relT_d = nc.dram_tensor("relT", [CH, 2 * sREL], bf, kind="ExternalInput")
    send_d = nc.dram_tensor("send_b", [sBAGS, sREL], f32, kind="ExternalInput")
    tgt_d = nc.dram_tensor("tgt", [sBAGS, sREL], f32, kind="ExternalInput")
    out_lg = nc.dram_tensor("out_logits", [sBAGS, sREL], f32, kind="ExternalOutput")
    out_ls = nc.dram_tensor("out_loss", [1, 1], f32, kind="ExternalOutput")
    dbg = bool(sizes and sizes.get("debug"))
    if dbg:
        out_x = nc.dram_tensor("out_x", [XK, sTOK], bf, kind="ExternalOutput")
        out_tup = nc.dram_tensor("out_tup", [CH, 2 * sT * sS], bf, kind="ExternalOutput")
        out_H = nc.dram_tensor("out_H", [CH, 2 * sS], f32, kind="ExternalOutput")
        out_row = nc.dram_tensor("out_row", [1, sTOK], f32, kind="ExternalOutput")
        out_al = nc.dram_tensor("out_al", [1, sTOK], f32, kind="ExternalOutput")
        out_abc = nc.dram_tensor("out_abc", [CH, sTOK], f32, kind="ExternalOutput")

    SCH = 64  # attention s-chunk
    n_sch = (sS + SCH - 1) // SCH

    with tile.TileContext(nc) as tc:
        with tc.tile_pool(name="const", bufs=1) as cp:
            # ---- load constants ----
            idn = cp.tile([128, 128], f32)
            make_identity(nc, idn[:])
            ids_sb = cp.tile([128, GT], i32)
            nc.sync.dma_start(out=ids_sb[:], in_=ids_d[:])
            pos_sb = cp.tile([128, GT * 10], f32)
            nc.sync.dma_start(out=pos_sb[:], in_=pos_d[:])
            wih = {}
            whh = {}
            brz = {}
            bhn = {}
            bin_ = {}
            for d in "fb":
                wih[d] = cp.tile([XK, G3], bf, name=f"wih{d}")
                nc.sync.dma_start(out=wih[d][:], in_=wih_d[d][:])
                whh[d] = cp.tile([CH, 2 * G3], bf, name=f"whh{d}")
                nc.sync.dma_start(out=whh[d][:], in_=whh_d[d][:])
                bhn[d] = cp.tile([CH, 2], f32, name=f"bhn{d}")
                nc.sync.dma_start(out=bhn[d][:], in_=bhn_d[d][:])
            aw = cp.tile([CH, 2], bf)
            nc.sync.dma_start(out=aw[:], in_=aw_d[:])
            sena = cp.tile([CH, 2], f32)
            nc.sync.dma_start(out=sena[:], in_=sena_d[:])
            senr = cp.tile([CH, 2], bf)
            nc.sync.dma_start(out=senr[:], in_=senr_d[:])
            relT = cp.tile([CH, 2 * sREL], bf)
            nc.sync.dma_start(out=relT[:], in_=relT_d[:])
            send_sb = cp.tile([sBAGS, sREL], f32)
            nc.sync.dma_start(out=send_sb[:], in_=send_d[:])
            tgt_sb = cp.tile([sBAGS, sREL], f32)
            nc.sync.dma_start(out=tgt_sb[:], in_=tgt_d[:])

            with tc.tile_pool(name="big", bufs=1) as bigp:
                tup = bigp.tile([CH, 2, sT, sS], bf, tag="tup")

                with tc.tile_pool(name="xpool", bufs=1) as xp_:
                    x_T = xp_.tile([XK, sTOK], bf, tag="xT")

                    # ================= phase 1: gather + transpose =============
                    with tc.tile_pool(name="gat", bufs=4) as gp, \
                         tc.tile_pool(name="trp", bufs=4, space="PSUM") as trp:
                        for g in range(GT):
                            n_tok = min(128, sTOK - g * 128)
                            et = gp.tile([128, 64], f32, tag="emb")
                            nc.vector.memset(et[:, 61:64], 0.0)
                            nc.vector.memset(et[:, 60:61], 1.0)
                            if n_tok < 128:
                                nc.vector.memset(et[:, 0:60], 0.0)
                            nc.gpsimd.indirect_dma_start(
                                out=et[:n_tok, 0:WORD_D],
                                out_offset=None,
                                in_=emb_d[:],
                                in_offset=bass.IndirectOffsetOnAxis(
                                    ap=ids_sb[:n_tok, g:g + 1], axis=0),
                            )
                            nc.vector.tensor_copy(
                                out=et[:, WORD_D:WORD_D + 10],
                                in_=pos_sb[:, g * 10:(g + 1) * 10])
                            pt = trp.tile([64, 128], f32, tag="tr")
                            nc.tensor.transpose(out=pt[:], in_=et[:], identity=idn[:])
                            if g % 2 == 0:
                                nc.vector.tensor_copy(
                                    out=x_T[:, g * 128:g * 128 + n_tok],
                                    in_=pt[0:XK, 0:n_tok])
                            else:
                                nc.scalar.copy(
                                    out=x_T[:, g * 128:g * 128 + n_tok],
                                    in_=pt[0:XK, 0:n_tok])

                    if dbg:
                        nc.sync.dma_start(out=out_x[:], in_=x_T[:])
                    # x viewed [XK, T, S] (t-major token order)
                    x3 = x_T[:].rearrange("p (t s) -> p t s", s=sS)
                    tup4 = tup[:]

                    # ===== phase 2: fwd+bwd GRU, interleaved chains ============
                    # fwd state in ping-pong tiles, copied into tup[t]; bwd
                    # state for t_phys >= HT parks in hb_half (doubling as
                    # its state storage); for t_phys < HT it adds into tup
                    # directly (fwd wrote those slots earlier in the loop).
                    HT = sT - sT // 2
                    with tc.tile_pool(name="gru", bufs=2) as grup, \
                         tc.tile_pool(name="gruh", bufs=1) as ghp, \
                         tc.tile_pool(name="gruz", bufs=1) as gzp, \
                         tc.tile_pool(name="gps", bufs=1, space="PSUM") as psp:
                        zeros = gzp.tile([CH, 2, sS], bf)
                        nc.vector.memset(zeros[:], 0.0)
                        hb_half = ghp.tile([CH, 2, HT, sS], bf)

                        def gru_step(d, t_phys, h_prev, dest):
                            """one GRU cell step; h_prev/dest = [CH,2,S] bf16 APs."""
                            x_t = x3[:, t_phys, :]
                            # per-dir 4-bank psum tile: slices 0-3 hold the
                            # r/z preacts; after sigma_r drains slices 0-1 the
                            # n-gate reuses them (start=True reopens). Dirs
                            # are fully decoupled (4+4 = 8 banks).
                            ps = psp.tile([CH, 4, sS], f32, tag="ps" + d, bufs=1,
                                          padded_shape=[CH, 4, 512])
                            for mc in range(4):
                                lhs_x = wih[d][:, mc * CH:(mc + 1) * CH]
                                lhs_h0 = whh[d][:, mc * CH:(mc + 1) * CH]
                                lhs_h1 = whh[d][:, G3 + mc * CH:G3 + (mc + 1) * CH]
                                nc.tensor.matmul(ps[:, mc, :], lhsT=lhs_x,
                                                 rhs=x_t, start=True, stop=False)
                                nc.tensor.matmul(ps[:, mc, :], lhsT=lhs_h0,
                                                 rhs=h_prev[:, 0, :],
                                                 start=False, stop=False)
                                nc.tensor.matmul(ps[:, mc, :], lhsT=lhs_h1,
                                                 rhs=h_prev[:, 1, :],
                                                 start=False, stop=True)
                            rzt = grup.tile([CH, 4, sS], bf, tag="rz")
                            # r first (on the n-gate critical path), z later
                            nc.scalar.activation(out=rzt[:, 0:2, :],
                                                 in_=ps[:, 0:2, :],
                                                 func=Act.Sigmoid)
                            nc.scalar.activation(out=rzt[:, 2:4, :],
                                                 in_=ps[:, 2:4, :],
                                                 func=Act.Sigmoid)
                            for c in range(2):
                                mc = 4 + c
                                lhs_x = wih[d][:, mc * CH:(mc + 1) * CH]
                                lhs_h0 = whh[d][:, mc * CH:(mc + 1) * CH]
                                lhs_h1 = whh[d][:, G3 + mc * CH:G3 + (mc + 1) * CH]
                                nc.tensor.matmul(ps[:, c, :], lhsT=lhs_h0,
                                                 rhs=h_prev[:, 0, :],
                                                 start=True, stop=False)
                                nc.tensor.matmul(ps[:, c, :], lhsT=lhs_h1,
                                                 rhs=h_prev[:, 1, :],
                                                 start=False, stop=True)
                                # psum <- (hn + b_hn) * r   (in-place DVE RMW)
                                nc.vector.scalar_tensor_tensor(
                                    out=ps[:, c, :], in0=ps[:, c, :],
                                    scalar=bhn[d][:, c:c + 1],
                                    in1=rzt[:, c, :], op0=Alu.add, op1=Alu.mult)
                                # xn (+b_in via the ones row) accumulates on
                                # top: has_written bits from the h-matmuls
                                # survive the DVE overwrite, so flags=0x0 adds.
                                nc.tensor.matmul(ps[:, c, :], lhsT=lhs_x,
                                                 rhs=x_t, start=False, stop=True,
                                                 skip_group_check=True)
                            nt = grup.tile([CH, 2, sS], bf, tag="n")
                            nc.scalar.activation(out=nt[:], in_=ps[:, 0:2, :],
                                                 func=Act.Tanh)
                            # h' = n + z*(h-n)
                            dt = grup.tile([CH, 2, sS], bf, tag="d")
                            nc.gpsimd.tensor_tensor(out=dt[:], in0=h_prev,
                                                    in1=nt[:], op=Alu.subtract)
                            nc.vector.tensor_tensor(out=dt[:], in0=dt[:],
                                                    in1=rzt[:, 2:4, :],
                                                    op=Alu.mult)
                            nc.vector.tensor_tensor(out=dest, in0=dt[:],
                                                    in1=nt[:], op=Alu.add)

                        hf_prev = None
                        hb_prev = None
                        for t in range(sT):
                            tp = sT - 1 - t
                            hf_new = grup.tile([CH, 2, sS], bf, tag="hf")
                            gru_step("f", t, zeros[:] if t == 0 else hf_prev[:],
                                     hf_new[:])
                            nc.gpsimd.tensor_copy(out=tup4[:, :, t, :],
                                                   in_=hf_new[:])
                            hf_prev = hf_new
                            if tp >= HT:
                                bdest = hb_half[:, :, tp - HT, :]
                                gru_step("b", tp,
                                         zeros[:] if t == 0 else hb_prev,
                                         bdest)
                                hb_prev = bdest
                            else:
                                hb_new = grup.tile([CH, 2, sS], bf, tag="hb")
                                gru_step("b", tp, hb_prev, hb_new[:])
                                # fwd wrote tup[tp] at iteration tp < t: safe
                                nc.vector.tensor_tensor(
                                    out=tup4[:, :, tp, :],
                                    in0=tup4[:, :, tp, :], in1=hb_new[:],
                                    op=Alu.add)
                                hb_prev = hb_new[:]
                        # batched add of the parked first-half backward states
                        for c in range(2):
                            nc.vector.tensor_tensor(
                                out=tup4[:, c, HT:sT, :],
                                in0=tup4[:, c, HT:sT, :],
                                in1=hb_half[:, c, :, :], op=Alu.add)

                if dbg:
                    nc.sync.dma_start(
                        out=out_tup[:],
                        in_=tup4.rearrange("p c t s -> p (c t s)"))
                # ================= phase 4: word attention =================
                H = gzp_tile = None
                with tc.tile_pool(name="att", bufs=2) as ap_, \
                     tc.tile_pool(name="attrow", bufs=1) as rp, \
                     tc.tile_pool(name="atth", bufs=1) as hp, \
                     tc.tile_pool(name="attdr", bufs=2, space="DRAM") as drp, \
                     tc.tile_pool(name="attps", bufs=4, space="PSUM") as scp:
                    H = hp.tile([CH, 2, sS], f32)
                    for sc_i in range(n_sch):
                        s0 = sc_i * SCH
                        ns = min(SCH, sS - s0)
                        th = ap_.tile([CH, 2, sT, SCH], bf, tag="xT")  # reuse xT slot
                        nc.scalar.activation(
                            out=th[:, :, :, 0:ns],
                            in_=tup4[:, :, :, s0:s0 + ns], func=Act.Tanh)
                        # scores: M=1 matmuls over s-groups of 7 (N=7*T<=512)
                        row = rp.tile([1, SCH * sT], f32, tag="row")
                        th_v = th[:].rearrange("p c t s -> p c s t")
                        SG = max(1, 512 // sT)
                        n_sg = (ns + SG - 1) // SG
                        for sg in range(n_sg):
                            a0 = sg * SG
                            na = min(SG, ns - a0)
                            sps = scp.tile([1, SG * sT], f32, tag="sc")
                            nc.tensor.matmul(
                                sps[:, 0:na * sT],
                                lhsT=aw[:, 0:1],
                                rhs=th_v[:, 0, a0:a0 + na, :],
                                start=True, stop=False)
                            nc.tensor.matmul(
                                sps[:, 0:na * sT],
                                lhsT=aw[:, 1:2],
                                rhs=th_v[:, 1, a0:a0 + na, :],
                                start=False, stop=True)
                            eng = nc.vector if sg % 2 == 0 else nc.scalar
                            if sg % 2 == 0:
                                nc.vector.tensor_copy(out=row[:, a0 * sT:(a0 + na) * sT],
                                                      in_=sps[:, 0:na * sT])
                            else:
                                nc.scalar.copy(out=row[:, a0 * sT:(a0 + na) * sT],
                                               in_=sps[:, 0:na * sT])
                        if dbg:
                            nc.sync.dma_start(
                                out=out_row[:, s0 * sT:(s0 + ns) * sT],
                                in_=row[:, 0:ns * sT])
                        # softmax in [s-part, t] layout (via DRAM for clean deps)
                        srow = drp.tile([SCH, sT], f32, tag="srow")
                        nc.sync.dma_start(
                            out=srow[0:ns, :].rearrange("s t -> (s t)").unsqueeze(0),
                            in_=row[0:1, 0:ns * sT])
                        sp_t = ap_.tile([SCH, sT], f32, tag="spt")
                        nc.sync.dma_start(out=sp_t[0:ns, :], in_=srow[0:ns, :])
                        mx = ap_.tile([SCH, 2], f32, tag="mx")
                        nc.vector.tensor_reduce(out=mx[0:ns, 0:1], in_=sp_t[0:ns, :],
                                                axis=mybir.AxisListType.X, op=Alu.max)
                        nc.vector.tensor_scalar_mul(mx[0:ns, 1:2], mx[0:ns, 0:1], -1.0)
                        den = ap_.tile([SCH, 2], f32, tag="den")
                        nc.scalar.activation(
                            out=sp_t[0:ns, :], in_=sp_t[0:ns, :], func=Act.Exp,
                            bias=mx[0:ns, 1:2], accum_out=den[0:ns, 0:1])
                        nc.vector.reciprocal(out=den[0:ns, 1:2], in_=den[0:ns, 0:1])
                        nc.vector.tensor_scalar(
                            out=sp_t[0:ns, :], in0=sp_t[0:ns, :],
                            scalar1=den[0:ns, 1:2], scalar2=None, op0=Alu.mult)
                        # alpha back to row layout, then broadcast across partitions
                        astg = drp.tile([SCH, sT], f32, tag="astg")
                        nc.sync.dma_start(out=astg[0:ns, :], in_=sp_t[0:ns, :])
                        abc = ap_.tile([CH, SCH * sT], f32, tag="abc", bufs=1)
                        nc.sync.dma_start(
                            out=abc[:, 0:ns * sT],
                            in_=astg[0:ns, :].rearrange("s t -> (s t)")
                                .unsqueeze(0).to_broadcast([CH, ns * sT]))
                        if dbg:
                            nc.sync.dma_start(
                                out=out_abc[:, s0 * sT:(s0 + ns) * sT],
                                in_=abc[:, 0:ns * sT])
                        # weighted pool: w = tup * alpha ; H = sum_t w
                        wv = ap_.tile([CH, 2, SCH, sT], bf, tag="wv", bufs=1)
                        abc_v = abc[:, 0:ns * sT].rearrange("p (s t) -> p s t", t=sT)
                        nc.vector.tensor_tensor(
                            out=wv[:, :, 0:ns, :],
                            in0=tup4[:, :, :, s0:s0 + ns].rearrange("p c t s -> p c s t"),
                            in1=abc_v.unsqueeze(1).broadcast_to([CH, 2, ns, sT]),
                            op=Alu.mult)
                        nc.vector.tensor_reduce(
                            out=H[:, :, s0:s0 + ns], in_=wv[:, :, 0:ns, :],
                            axis=mybir.AxisListType.X, op=Alu.add)

                    if dbg:
                        nc.sync.dma_start(
                            out=out_H[:], in_=H[:].rearrange("p c s -> p (c s)"))
                    # ================= phase 5: bag attention + logits + loss ====
                    Ha = ap_.tile([CH, 2, sS], bf, tag="xT")
                    for c in range(2):
                        nc.vector.tensor_scalar(
                            out=Ha[:, c, :], in0=H[:, c, :],
                            scalar1=sena[:, c:c + 1], scalar2=None, op0=Alu.mult)
                    eps = scp.tile([1, sS], f32, tag="sc")
                    nc.tensor.matmul(eps[:], lhsT=senr[:, 0:1], rhs=Ha[:, 0, :],
                                     start=True, stop=False)
                    nc.tensor.matmul(eps[:], lhsT=senr[:, 1:2], rhs=Ha[:, 1, :],
                                     start=False, stop=True)
                    erow = rp.tile([1, sS], f32, tag="row")
                    nc.vector.tensor_copy(out=erow[:], in_=eps[:])
                    estg = drp.tile([sBAGS, sNSENT], f32, tag="estg")
                    nc.sync.dma_start(
                        out=estg[:].rearrange("b n -> (b n)").unsqueeze(0),
                        in_=erow[0:1, :])
                    ep = ap_.tile([sBAGS, sNSENT], f32, tag="ep")
                    nc.sync.dma_start(out=ep[:], in_=estg[:])
                    mx2 = ap_.tile([sBAGS, 2], f32, tag="mx")
                    nc.vector.tensor_reduce(out=mx2[:, 0:1], in_=ep[:],
                                            axis=mybir.AxisListType.X, op=Alu.max)
                    nc.vector.tensor_scalar_mul(mx2[:, 1:2], mx2[:, 0:1], -1.0)
                    den2 = ap_.tile([sBAGS, 2], f32, tag="den")
                    nc.scalar.activation(out=ep[:], in_=ep[:], func=Act.Exp,
                                         bias=mx2[:, 1:2], accum_out=den2[:, 0:1])
                    nc.vector.reciprocal(out=den2[:, 1:2], in_=den2[:, 0:1])
                    ap_bag = ap_.tile([sBAGS, sNSENT], f32, tag="apb")
                    nc.vector.tensor_scalar(out=ap_bag[:], in0=ep[:],
                                            scalar1=den2[:, 1:2], scalar2=None,
                                            op0=Alu.mult)
                    astg2 = drp.tile([sBAGS, sNSENT], f32, tag="astg2")
                    nc.sync.dma_start(out=astg2[:], in_=ap_bag[:])
                    Abc = ap_.tile([CH, sS], f32, tag="Abc")
                    nc.sync.dma_start(
                        out=Abc[:],
                        in_=astg2[:].rearrange("b n -> (b n)")
                            .unsqueeze(0).to_broadcast([CH, sS]))
                    w2 = ap_.tile([CH, 2, sBAGS, sNSENT], f32, tag="w2")
                    nc.vector.tensor_tensor(
                        out=w2[:],
                        in0=H[:].rearrange("p c (b n) -> p c b n", n=sNSENT),
                        in1=Abc[:].rearrange("p (b n) -> p b n", n=sNSENT)
                            .unsqueeze(1).broadcast_to([CH, 2, sBAGS, sNSENT]),
                        op=Alu.mult)
                    Sv = ap_.tile([CH, 2, sBAGS], f32, tag="Sv")
                    nc.vector.tensor_reduce(out=Sv[:], in_=w2[:],
                                            axis=mybir.AxisListType.X, op=Alu.add)
                    Svb = ap_.tile([CH, 2, sBAGS], bf, tag="Svb")
                    nc.vector.tensor_copy(out=Svb[:], in_=Sv[:])
                    lgps = scp.tile([sBAGS, sREL], f32, tag="lg", bufs=1)
                    nc.tensor.matmul(lgps[:], lhsT=Svb[:, 0, :], rhs=relT[:, 0:sREL],
                                     start=True, stop=False)
                    nc.tensor.matmul(lgps[:], lhsT=Svb[:, 1, :], rhs=relT[:, sREL:2 * sREL],
                                     start=False, stop=True)
                    lg_sb = ap_.tile([sBAGS, sREL], f32, tag="lgs")
                    nc.vector.tensor_tensor(out=lg_sb[:], in0=lgps[:], in1=send_sb[:],
                                            op=Alu.add)
                    nc.sync.dma_start(out=out_lg[:], in_=lg_sb[:])
                    # BCE: relu(l) - l*tgt + log(1+exp(-|l|))
                    t1 = ap_.tile([sBAGS, sREL], f32, tag="t1")
                    nc.scalar.activation(out=t1[:], in_=lg_sb[:], func=Act.Relu)
                    t2 = ap_.tile([sBAGS, sREL], f32, tag="t2")
                    nc.vector.tensor_tensor(out=t2[:], in0=lg_sb[:], in1=tgt_sb[:],
                                            op=Alu.mult)
                    nc.vector.tensor_tensor(out=t1[:], in0=t1[:], in1=t2[:],
                                            op=Alu.subtract)
                    t3 = ap_.tile([sBAGS, sREL], f32, tag="t3")
                    nc.scalar.activation(out=t3[:], in_=lg_sb[:], func=Act.Abs)
                    nc.scalar.activation(out=t3[:], in_=t3[:], func=Act.Exp,
                                         scale=-1.0)
                    nc.vector.tensor_scalar(out=t3[:], in0=t3[:], scalar1=1.0,
                                            scalar2=None, op0=Alu.add)
                    nc.scalar.activation(out=t3[:], in_=t3[:], func=Act.Ln)
                    nc.vector.tensor_tensor(out=t1[:], in0=t1[:], in1=t3[:], op=Alu.add)
                    br = ap_.tile([sBAGS, 2], f32, tag="br")
                    nc.vector.tensor_reduce(out=br[:, 0:1], in_=t1[:],
                                            axis=mybir.AxisListType.X, op=Alu.add)
                    nc.vector.tensor_scalar_mul(br[:, 1:2], br[:, 0:1], 1.0 / sREL)
                    ones = ap_.tile([sBAGS, 1], f32, tag="ones")
                    nc.vector.memset(ones[:], 1.0)
                    lsps = scp.tile([1, 1], f32, tag="ls", bufs=1)
                    nc.tensor.matmul(lsps[:], lhsT=br[:, 1:2], rhs=ones[:],
                                     start=True, stop=True)
                    ls_sb = ap_.tile([1, 1], f32, tag="lss")
                    nc.vector.tensor_copy(out=ls_sb[:], in_=lsps[:])
                    nc.sync.dma_start(out=out_ls[:], in_=ls_sb[:])

    nc.compile()
    return nc, z


def _host_prep(inputs, core):
    """Build the per-core in_map (numpy host prep only: sharding, index reshape,
    dtype casts, weight layout)."""
    def f32(x):
        return np.ascontiguousarray(np.asarray(x, np.float32))

    E = f32(inputs["embedding"])
    PEm = f32(inputs["pos_embedding"])
    tok = np.asarray(inputs["sentence_bag"]).reshape(B * NSENT, T)
    p1 = np.asarray(inputs["pos1_bag"]).reshape(B * NSENT, T)
    p2 = np.asarray(inputs["pos2_bag"]).reshape(B * NSENT, T)
    lab = np.asarray(inputs["label_bag"]).reshape(B)

    s0 = core * S
    tok_c = np.ascontiguousarray(tok[s0:s0 + S].T).reshape(-1).astype(np.int64)  # t-major
    p1_c = np.ascontiguousarray(p1[s0:s0 + S].T).reshape(-1).astype(np.int64)
    p2_c = np.ascontiguousarray(p2[s0:s0 + S].T).reshape(-1).astype(np.int64)
    GT = TOK // 128
    ids = np.ascontiguousarray(tok_c.reshape(GT, 128).T).astype(np.int32)
    pos_pay = np.concatenate([PEm[p1_c], PEm[p2_c]], -1)  # [TOK, 10]
    pos_pay = np.ascontiguousarray(
        pos_pay.reshape(GT, 128, 10).transpose(1, 0, 2).reshape(128, GT * 10)
    ).astype(np.float32)

    m = {"emb_w": E, "ids": ids, "pos_pay": pos_pay}
    for d, sfx in (("f", "_f"), ("b", "_b")):
        Wih = f32(inputs["W_ih" + sfx])       # [600, 60]
        Whh = f32(inputs["W_hh" + sfx])       # [600, 200]
        bih = f32(inputs["b_ih" + sfx])
        bhh = f32(inputs["b_hh" + sfx])
        wih = np.empty((IN_D + 1, G3), np.float32)
        wih[:IN_D] = Wih.T
        # bias row (multiplied by the constant-1 x row): r,z get b_ih+b_hh;
        # n gets b_ih only (b_hh_n lives inside the r* product).
        wih[IN_D, 0:400] = bih[0:400] + bhh[0:400]
        wih[IN_D, 400:600] = bih[400:600]
        m[f"wih_{d}"] = wih.astype(BF16)
        whh = np.empty((CH, 2 * G3), np.float32)
        for kc in range(2):
            whh[:, kc * G3:(kc + 1) * G3] = Whh.T[kc * CH:(kc + 1) * CH, :]
        m[f"whh_{d}"] = whh.astype(BF16)
        m[f"bhn_{d}"] = np.stack([bhh[400:500], bhh[500:600]], -1).astype(np.float32)
    awf = f32(inputs["attention_w"])[:, 0]
    m["aw"] = np.stack([awf[:CH], awf[CH:]], -1).astype(BF16)
    sa = f32(inputs["sen_a"])
    m["sena"] = np.stack([sa[:CH], sa[CH:]], -1).astype(np.float32)
    sr = f32(inputs["sen_r"])[:, 0]
    m["senr"] = np.stack([sr[:CH], sr[CH:]], -1).astype(BF16)
    RE = f32(inputs["relation_embedding"])    # [REL, HID]
    relT = np.empty((CH, 2 * REL), np.float32)
    for c in range(2):
        relT[:, c * REL:(c + 1) * REL] = RE[:, c * CH:(c + 1) * CH].T
    m["relT"] = relT.astype(BF16)
    m["send_b"] = np.broadcast_to(f32(inputs["sen_d"]), (BAGS, REL)).copy()
    lab_c = lab[core * BAGS:(core + 1) * BAGS]
    m["tgt"] = np.eye(REL, dtype=np.float32)[lab_c]
    return m


_CACHE = {}


def kernel(**inputs):
    from concourse.bass_utils import run_bass_kernel_spmd

    if "nc" not in _CACHE:
        nc, _ = _build()
        _CACHE["nc"] = nc
    nc = _CACHE["nc"]
    in_maps = [_host_prep(inputs, c) for c in range(NCORES)]
    res = run_bass_kernel_spmd(nc, in_maps, core_ids=list(range(NCORES)))
    outs = res.results
    logits = np.concatenate([o["out_logits"] for o in outs], 0).astype(np.float32)
    loss = np.float32(sum(float(o["out_loss"][0, 0]) for o in outs))
    return loss, logits
